# revision 21
# baseline (speedup 1.0000x reference)
"""Causal self-attention (B=2, T=2048, C=1024, H=16) on 8 TRN2 NeuronCores.

Sharding: core = (batch b, head-group hg) with b in {0,1}, hg in {0..3};
each core computes Q/K/V projections and attention for its 4 heads on its
batch, plus the row-parallel slice of the output projection. The host sums
the 4 per-core partial projections per batch and adds the output bias.

Device algorithm (all layouts transposed so softmax needs no on-chip
transposes):
  - Q^T, K^T [dd, t] and V [t, dd] via fp32r matmuls (contraction over C).
  - S^T[s, t] = K^T.T-free matmul pair, 2 heads row-packed in the 128-wide
    PE array (K=64 each at row offsets 0/64).
  - exp on ScalarE straight out of PSUM (scale=1/sqrt(d) folded in); causal
    masking = one additive 128x128 band on diagonal blocks + trimming the
    AV matmul's moving range; softmax denominators from an all-ones column
    appended to V (M=65 matmul); normalization deferred to after AV.
  - y_partial[t, e] = O^T.T @ Wp^T slice, accumulated over the 2 dd-chunks.
"""

import math
from functools import lru_cache

import ml_dtypes
import numpy as np

import concourse.bass as bass
import concourse.mybir as mybir
from concourse import bacc
import concourse.tile as tile
from concourse import bass_utils

F32 = mybir.dt.float32
F32R = mybir.dt.float32r
BF16 = mybir.dt.bfloat16
EXP = mybir.ActivationFunctionType.Exp

B, T, C, H = 2, 2048, 1024, 16
NCORES = 8
NH = 4            # heads per core
D = C // H        # 64
DD = NH * D       # 256 channels per core
P = 128
TG = 512          # t-group width (matmul moving dim)
NG = T // TG      # 4
NT = T // P       # 16 s-chunks
CCH = C // P      # 8 contraction chunks
NEG = -8.0e6      # pre-scale additive mask; *0.125 = -1e6 like the reference

LAST_RESULTS = None  # BassKernelResults of the most recent run (for test.py)


def build_program(apply_kbias: bool, general_mask: bool) -> bass.Bass:
    nc = bacc.Bacc("TRN2", target_bir_lowering=False, debug=False,
                   enable_asserts=False)

    xT = nc.dram_tensor("xT", [C, T], BF16, kind="ExternalInput").ap()
    wqT = nc.dram_tensor("wqT", [C, DD], BF16, kind="ExternalInput").ap()
    wkT = nc.dram_tensor("wkT", [C, DD], BF16, kind="ExternalInput").ap()
    wvT = nc.dram_tensor("wvT", [C, DD], BF16, kind="ExternalInput").ap()
    wpT = nc.dram_tensor("wpT", [DD, C], BF16, kind="ExternalInput").ap()
    bqk = nc.dram_tensor("bqk", [P, 4], F32, kind="ExternalInput").ap()
    bv_in = nc.dram_tensor("bv_sb", [P, DD], F32, kind="ExternalInput").ap()
    kbias_in = None
    if apply_kbias:
        kbias_in = nc.dram_tensor("kbias", [P, NT], F32, kind="ExternalInput").ap()
    band_in = maskT = None
    if general_mask:
        maskT = nc.dram_tensor("maskT", [T, T], F32, kind="ExternalInput").ap()
    else:
        band_in = nc.dram_tensor("band", [P, P], F32, kind="ExternalInput").ap()
    yp = nc.dram_tensor("yp", [T, C], F32, kind="ExternalOutput").ap()
    # DRAM scratch used to broadcast softmax reciprocal rows across
    # partitions (DMA from DRAM may use a 0-step partition dim; SBUF may not)
    rcd = nc.dram_tensor("rcd", [8, 2 * TG], F32, kind="Internal").ap()

    with tile.TileContext(nc) as tc:
        with tc.tile_pool(name="wts", bufs=1) as wts, \
             tc.tile_pool(name="xtp", bufs=1) as xtp, \
             tc.tile_pool(name="qkv", bufs=1) as qkv, \
             tc.tile_pool(name="otp", bufs=1) as otp, \
             tc.tile_pool(name="ptp", bufs=2) as ptp, \
             tc.tile_pool(name="asb", bufs=4) as asbp, \
             tc.tile_pool(name="rcp", bufs=2) as rcp, \
             tc.tile_pool(name="bcp", bufs=2) as bcp, \
             tc.tile_pool(name="tmp", bufs=2) as tmpp, \
             tc.tile_pool(name="ysb", bufs=2) as ysbp, \
             tc.tile_pool(name="mkp", bufs=2) as mkp, \
             tc.tile_pool(name="stp", bufs=2, space="PSUM") as stp, \
             tc.tile_pool(name="avp", bufs=2, space="PSUM") as avp, \
             tc.tile_pool(name="mmp", bufs=2, space="PSUM") as mmp:

            # Pre-load the one ACT table set containing BOTH Exp and Ln so
            # the act-table pass doesn't thrash between per-function sets
            # (measured 17 TABLE_LOADs / 22us without this).
            from concourse.hw_specs import get_activation_tables
            tables = get_activation_tables(nc.m.arch)
            combined_id = list(tables).index("natural_log_exp_and_others")
            nc.scalar.add_instruction(mybir.InstLoadActFuncSet(
                name=nc.get_next_instruction_name(), ins=[], outs=[],
                act_func_set_id=combined_id))

            # ---- input DMAs ----
            wq = [wts.tile([P, DD], BF16, name=f"wq{c}") for c in range(CCH)]
            wk = [wts.tile([P, DD], BF16, name=f"wk{c}") for c in range(CCH)]
            wv = [wts.tile([P, DD], BF16, name=f"wv{c}") for c in range(CCH)]
            xt = [xtp.tile([P, T], BF16, name=f"xt{c}") for c in range(CCH)]
            bqk_t = wts.tile([P, 4], F32, name="bqk_t")
            nc.sync.dma_start(out=bqk_t, in_=bqk)
            # x + K/Q weights first: the projection chains consume xt[c] in
            # order, so interleave per-chunk to start the PE earliest.
            for c in range(CCH):
                nc.sync.dma_start(out=wk[c], in_=wkT[c * P:(c + 1) * P, :])
                nc.sync.dma_start(out=wq[c], in_=wqT[c * P:(c + 1) * P, :])
                nc.sync.dma_start(out=xt[c], in_=xT[c * P:(c + 1) * P, :])
            for c in range(CCH):
                nc.sync.dma_start(out=wv[c], in_=wvT[c * P:(c + 1) * P, :])
            wp = [wts.tile([P, C], BF16, name=f"wp{i}") for i in range(2)]
            for i in range(2):
                nc.sync.dma_start(out=wp[i], in_=wpT[i * P:(i + 1) * P, :])
            bv_sb = wts.tile([P, DD], F32, name="bv_t")
            nc.sync.dma_start(out=bv_sb, in_=bv_in)
            if band_in is not None:
                band_t = wts.tile([P, P], F32, name="band_t")
                nc.sync.dma_start(out=band_t, in_=band_in)
            if kbias_in is not None:
                kbias_t = wts.tile([P, NT], F32, name="kbias_t")
                nc.sync.dma_start(out=kbias_t, in_=kbias_in)

            qt = [qkv.tile([P, T], BF16, name=f"qt{i}") for i in range(2)]
            kt = [qkv.tile([P, T], BF16, name=f"kt{i}") for i in range(2)]
            vaug = [qkv.tile([P, NH * (D + 1)], BF16, name=f"vaug{j}")
                    for j in range(NT)]
            ot = [otp.tile([P, T], BF16, name=f"ot{i}") for i in range(2)]

            # ---- QKV projections ----
            # During QKV the attention PSUM pools are idle; rotating chains
            # across all three pools lets 6 accumulation chains run while
            # xT streams in, instead of 2.
            qkv_ps_state = [0]

            def qkv_ps():
                k = qkv_ps_state[0] % 4
                qkv_ps_state[0] += 1
                if k < 2:
                    return mmp.tile([P, TG], F32, name="mm", tag="mm")
                elif k == 2:
                    return stp.tile([P, 2 * TG], F32, name="st",
                                    tag="st")[:, 0:TG]
                else:
                    return avp.tile([P, TG], F32, name="av", tag="av")

            def qk_proj(i, dst, w, bias_col):
                for tg in range(NG):
                    ps = qkv_ps()
                    for c in range(CCH):
                        nc.tensor.matmul(
                            ps,
                            lhsT=(w[c][:, i * P:(i + 1) * P]),
                            rhs=(xt[c][:, tg * TG:(tg + 1) * TG]),
                            start=(c == 0), stop=(c == CCH - 1))
                    nc.vector.tensor_scalar_add(
                        dst[:, tg * TG:(tg + 1) * TG], ps,
                        bqk_t[:, bias_col:bias_col + 1])

            def v_proj(j):
                ps = qkv_ps()
                for c in range(CCH):
                    nc.tensor.matmul(
                        ps[:, :DD],
                        lhsT=(xt[c][:, j * P:(j + 1) * P]),
                        rhs=(wv[c]),
                        start=(c == 0), stop=(c == CCH - 1))
                vview = vaug[j].rearrange("p (h x) -> p h x", h=NH)
                # ones column (softmax denominator row): in0*0 + 1 -> f32r
                nc.vector.tensor_scalar(
                    vview[:, :, D:D + 1],
                    bv_sb.rearrange("p (h x) -> p h x", h=NH)[:, :, 0:1],
                    0.0, 1.0,
                    mybir.AluOpType.mult, mybir.AluOpType.add)
                nc.vector.tensor_add(
                    vview[:, :, 0:D],
                    ps[:, :DD].rearrange("p (h x) -> p h x", h=NH),
                    bv_sb.rearrange("p (h x) -> p h x", h=NH))


            # ---- attention ----
            def attn_block(i, g):
                # causal: only s-chunks on/below the diagonal contribute.
                # general mask: every s-chunk may contribute.
                nj = NT if general_mask else 4 * g + 4
                av = [avp.tile([P, TG], F32, name="av", tag="av")
                      for _ in range(2)]
                for j in range(nj):
                    r = j - 4 * g  # >=0 on diagonal blocks
                    st = stp.tile([P, 2 * TG], F32, name="st", tag="st")
                    for h in range(2):
                        nc.tensor.matmul(
                            st[:, h * TG:(h + 1) * TG],
                            lhsT=(kt[i][64 * h:64 * h + 64,
                                             j * P:(j + 1) * P]),
                            rhs=(qt[i][64 * h:64 * h + 64,
                                            g * TG:(g + 1) * TG]),
                            start=True, stop=True,
                            tile_position=(64 * h, 0))
                    if general_mask:
                        mk = mkp.tile([P, TG], F32, name="mk", tag="mk")
                        nc.sync.dma_start(
                            out=mk,
                            in_=maskT[j * P:(j + 1) * P, g * TG:(g + 1) * TG])
                        for h in range(2):
                            nc.vector.tensor_add(
                                st[:, h * TG:(h + 1) * TG],
                                st[:, h * TG:(h + 1) * TG], mk)
                    elif r >= 0:
                        for h in range(2):
                            sl = slice(h * TG + r * P, h * TG + (r + 1) * P)
                            nc.vector.tensor_add(st[:, sl], st[:, sl], band_t)
                    if apply_kbias:
                        for h in range(2):
                            nc.vector.tensor_scalar_add(
                                st[:, h * TG:(h + 1) * TG],
                                st[:, h * TG:(h + 1) * TG],
                                kbias_t[:, j:j + 1])
                    pt = ptp.tile([P, 2 * TG], BF16, name="pt", tag="pt")
                    nc.scalar.activation(pt, st, EXP, scale=1.0 / math.sqrt(D))
                    trim = r * P if (r > 0 and not general_mask) else 0
                    for h in range(2):
                        nc.tensor.matmul(
                            av[h][0:D + 1, trim:TG],
                            lhsT=(vaug[j][:, (2 * i + h) * (D + 1):
                                               (2 * i + h + 1) * (D + 1)]),
                            rhs=(pt[:, h * TG + trim:(h + 1) * TG]),
                            start=(j == 0), stop=(j == nj - 1),
                            skip_group_check=True)
                # one fast PSUM->SBUF copy per head releases the accumulator
                # bank; reciprocal = exp(-ln(sums)) on ScalarE (keeps the DVE
                # FIFO free), broadcast across partitions via a DRAM bounce.
                slot = i * NG + g
                asb = asbp.tile([D + 1, 2 * TG], F32, name="asb", tag="asb")
                nc.vector.tensor_copy(asb[:, 0:TG], av[0][0:D + 1, :])
                nc.vector.tensor_copy(asb[:, TG:2 * TG], av[1][0:D + 1, :])
                rc = rcp.tile([P, 2 * TG], F32, name="rc", tag="rc")
                nc.scalar.activation(
                    rc[D:D + 1, :], asb[D:D + 1, :],
                    mybir.ActivationFunctionType.Ln)
                nc.scalar.activation(
                    rc[D:D + 1, :], rc[D:D + 1, :],
                    EXP, scale=-1.0)
                nc.sync.dma_start(out=rcd[slot], in_=rc[D:D + 1, :])
                bc = bcp.tile([P, 2 * TG], F32, name="bc", tag="bc")
                bcast_src = bass.AP(
                    tensor=rcd.tensor, offset=rcd[slot].offset,
                    ap=[[0, D]] + list(rcd[slot].ap))
                nc.sync.dma_start(out=bc[0:D, :], in_=bcast_src)
                nc.vector.tensor_mul(
                    ot[i][0:D, g * TG:(g + 1) * TG],
                    asb[0:D, 0:TG], bc[0:D, 0:TG])
                tm = tmpp.tile([P, TG], BF16, name="tm", tag="tm")
                nc.vector.tensor_mul(tm[0:D, :], asb[0:D, TG:2 * TG],
                                     bc[0:D, TG:2 * TG])
                nc.sync.dma_start(
                    out=ot[i][64:128, g * TG:(g + 1) * TG],
                    in_=tm[0:D, :])

            def proj_block(tt, ec):
                ps = mmp.tile([P, TG], F32, name="mm", tag="mm")
                for i in range(2):
                    nc.tensor.matmul(
                        ps,
                        lhsT=(ot[i][:, tt * P:(tt + 1) * P]),
                        rhs=(wp[i][:, ec * TG:(ec + 1) * TG]),
                        start=(i == 0), stop=(i == 1))
                ysb = ysbp.tile([P, TG], F32, name="ysb", tag="ysb")
                nc.vector.tensor_copy(ysb, ps)
                nc.sync.dma_start(
                    out=yp[tt * P:(tt + 1) * P, ec * TG:(ec + 1) * TG],
                    in_=ysb)

            # Emission order drives Tile's scheduling priority: pair-0
            # inputs, then pair-0's biggest attention block interleaved with
            # pair-1's projections (keeps ScalarE exp-ing while the PE runs
            # QKV), then the rest in descending-g order so the last block
            # (and the projections serialized behind it) is the smallest.
            qk_proj(0, kt[0], wk, 2)
            qk_proj(0, qt[0], wq, 0)
            for j in range(NT):
                v_proj(j)
            attn_block(0, NG - 1)
            qk_proj(1, kt[1], wk, 3)
            qk_proj(1, qt[1], wq, 1)
            for g in range(NG - 1, -1, -1):
                if g != NG - 1:
                    attn_block(0, g)
                attn_block(1, g)
                for tt in range(4 * g, 4 * g + 4):
                    for ec in range(2):
                        proj_block(tt, ec)

    nc.compile()
    return nc


@lru_cache(maxsize=4)
def _program(apply_kbias: bool, general_mask: bool) -> bass.Bass:
    return build_program(apply_kbias, general_mask)


def _host_prep(inputs):
    x = np.asarray(inputs["x"], np.float32)
    Wq = np.asarray(inputs["Wq"], np.float32)
    bq = np.asarray(inputs["bq"], np.float32)
    Wk = np.asarray(inputs["Wk"], np.float32)
    bk = np.asarray(inputs["bk"], np.float32)
    Wv = np.asarray(inputs["Wv"], np.float32)
    bv = np.asarray(inputs["bv"], np.float32)
    Wp = np.asarray(inputs["Wp"], np.float32)
    attn_mask = np.asarray(inputs["attn_mask"])
    valid = np.asarray(inputs["valid_input_mask"])

    tril = np.tril(np.ones((T, T), attn_mask.dtype))
    causal = all(np.array_equal(attn_mask[b], tril) for b in range(B))
    kbias_all = (valid.astype(np.float32) - 1.0) * 1e6  # [B, T]
    apply_kbias = bool((valid == 0).any())

    band = np.where(np.arange(P)[:, None] <= np.arange(P)[None, :],
                    np.float32(0.0), np.float32(NEG))

    in_maps = []
    for core in range(NCORES):
        b, hg = divmod(core, 4)
        sl = slice(hg * DD, (hg + 1) * DD)
        m = {
            "xT": np.ascontiguousarray(x[b].T).astype(ml_dtypes.bfloat16),
            "wqT": np.ascontiguousarray(Wq[sl, :].T).astype(ml_dtypes.bfloat16),
            "wkT": np.ascontiguousarray(Wk[sl, :].T).astype(ml_dtypes.bfloat16),
            "wvT": np.ascontiguousarray(Wv[sl, :].T).astype(ml_dtypes.bfloat16),
            "wpT": np.ascontiguousarray(Wp[:, sl].T).astype(ml_dtypes.bfloat16),
            "bqk": np.ascontiguousarray(
                np.stack([bq[sl][:P], bq[sl][P:], bk[sl][:P], bk[sl][P:]], 1)),
            "bv_sb": np.ascontiguousarray(np.tile(bv[sl], (P, 1))),
        }
        if apply_kbias:
            m["kbias"] = np.ascontiguousarray(kbias_all[b].reshape(NT, P).T)
        if not causal:
            m["maskT"] = np.ascontiguousarray(
                (attn_mask[b].T.astype(np.float32) - 1.0) * (-NEG))
        else:
            m["band"] = band
        in_maps.append(m)
    return in_maps, apply_kbias, causal


def _run(inputs, trace=False, trace_cores=None):
    global LAST_RESULTS
    in_maps, apply_kbias, causal = _host_prep(inputs)
    nc = _program(apply_kbias, not causal)
    res = bass_utils.run_bass_kernel_spmd(
        nc, in_maps, core_ids=list(range(NCORES)), trace=trace,
        trace_cores=trace_cores)
    LAST_RESULTS = res

    bp = np.asarray(inputs["bp"], np.float32)
    y = np.zeros((B, T, C), np.float32)
    for core in range(NCORES):
        y[core // 4] += res.results[core]["yp"]
    y += bp[None, None, :]
    return y


def kernel(**inputs) -> np.ndarray:
    return _run(inputs)
